# revision 34
# baseline (speedup 1.0000x reference)
"""Expert-parallel MoE MLP (top-2 of 8 experts) on 8 TRN2 NeuronCores.

Strategy (expert-parallel, per sharding hint):
  - core e holds expert e's weights (w1[e], w2[e], host-pre-transposed, bf16)
  - host dispatches tokens by expert id into a COMPACT layout: core e's
    xT holds exactly its routed tokens (ordered by owned-segment, then
    owner, then token id), padded only at the tail to a multiple of 128.
    This minimizes matmul columns (vs block-padded dispatch).
  - core e computes y = [silu(x_e @ w1[e]^T) * c_e] @ w2[e]^T over
    512-column chunks (bf16 matmuls, fp32 accumulate).
  - mm2 output m-tiles are indirect-scattered into `sendbuf`, which holds
    the AllToAll wire layout: NSEG slot-segments, stored in REVERSE
    segment order (plus a leading trash region for pad rows) so that each
    scatter's AP byte-extent is a prefix that never overlaps the A2A read
    slices of earlier-fired segments (avoids false WAR serialization).
  - NSEG chunked AllToAlls fire as soon as the last m-tile holding each
    segment's tokens has been scattered; segment s holds the partial rows
    of owned-token m-tile s on every owner, so the owner-side combine
    (2 indirect row-gathers + add) runs right after each A2A lands,
    overlapped with remaining compute and later A2As.
  - 16 dummy matmuls at t=0 warm the PE HAM clock gate during the loads.
"""

import sys

sys.path.insert(0, "/opt/trn_rl_repo")

import numpy as np
import ml_dtypes

import concourse.bass as bass
import concourse.tile as tile
from concourse import bacc, mybir
from concourse.bass_utils import run_bass_kernel_spmd

S, DM, DF, E, TOPK = 4096, 1024, 2048, 8, 2
NCORES = 8
P = 128
OWN = S // NCORES  # tokens per owner core
# owned tokens per A2A segment. ncfw runs collectives serially at ~10us
# each regardless of size; boundaries are sized so the fire points land at
# compute m-tiles 2/3/5/6/7 — the serial ncfw chain then interleaves with
# the slowest core's final m-tiles and only the last small A2A is exposed.
SEGLENS = [192, 64, 128, 64, 64]
SEGSTART = [sum(SEGLENS[:s]) for s in range(len(SEGLENS))]
NSEG = len(SEGLENS)
assert sum(SEGLENS) == OWN
# combine gathers operate on <=128-lane columns: (seg, lane offset, width)
GCOLS = [
    (s, off, min(P, SEGLENS[s] - off))
    for s in range(NSEG)
    for off in range(0, SEGLENS[s], P)
]
NGC = len(GCOLS)

_PROGRAM_CACHE: dict = {}


def _chunks_of(ntok: int) -> list[tuple[int, int]]:
    """Split ntok into (start, size) chunks, each a multiple of 128, <= 512."""
    out, pos = [], 0
    while pos < ntok:
        sz = min(512, ntok - pos)
        out.append((pos, sz))
        pos += sz
    return out


def _emit(nc, tc, ctx, plan: dict):
    dt = mybir.dt
    ntok = plan["ntok"]
    SS = plan["SS"]  # per-seg slot-block size
    FI = plan["FI"]  # fire A2A-s after scatter of m-tile FI[s]
    nmt = ntok // P

    SCOPS = plan["SCOPS"]  # per-m-tile scatter ops: list of (tm, seg)
    n_ops = len(SCOPS)

    xT = nc.dram_tensor("xT", [DM, ntok], dt.bfloat16, kind="ExternalInput").ap()
    w1t = nc.dram_tensor("w1t", [DM, DF], dt.bfloat16, kind="ExternalInput").ap()
    w2t = nc.dram_tensor("w2t", [DF, DM], dt.bfloat16, kind="ExternalInput").ap()
    cv = nc.dram_tensor("cv", [ntok], dt.float32, kind="ExternalInput").ap()
    scat = nc.dram_tensor("scat", [P * n_ops], dt.int32, kind="ExternalInput").ap()
    g0 = nc.dram_tensor("g0", [P * NGC], dt.int32, kind="ExternalInput").ap()
    g1 = nc.dram_tensor("g1", [P * NGC], dt.int32, kind="ExternalInput").ap()
    yout = nc.dram_tensor("yout", [OWN, DM], dt.float32, kind="ExternalOutput").ap()
    # one sendbuf per segment: indirect-scatter writes are tracked
    # conservatively (whole tensor), so per-seg tensors keep seg-s scatters
    # independent of other segments' in-flight AllToAll reads. Last P rows
    # of each are a trash region for pad tokens.
    sb = [
        nc.dram_tensor(f"send{s}", [8 * SS[s] + P, DM], dt.bfloat16).ap()
        for s in range(NSEG)
    ]
    recv = [
        nc.dram_tensor(f"recv{s}", [8 * SS[s] + 1, DM], dt.bfloat16).ap()
        for s in range(NSEG)
    ]

    dsend = nc.dram_tensor("dsend", [NCORES, 64], dt.bfloat16).ap()
    drecv = nc.dram_tensor("drecv", [NCORES, 64], dt.bfloat16).ap()

    wpool = ctx.enter_context(tc.tile_pool(name="w", bufs=1))
    hpool = ctx.enter_context(tc.tile_pool(name="h", bufs=34))
    ypool = ctx.enter_context(tc.tile_pool(name="y", bufs=10))
    gpool = ctx.enter_context(tc.tile_pool(name="g", bufs=2))
    phpool = ctx.enter_context(tc.tile_pool(name="ph", bufs=2, space="PSUM"))
    pypool = ctx.enter_context(tc.tile_pool(name="py", bufs=4, space="PSUM"))
    pwpool = ctx.enter_context(tc.tile_pool(name="pw", bufs=1, space="PSUM"))

    # ---- dummy collective at max priority: absorbs per-core start skew
    # early (overlapped with loads/compute) so the real A2As see short
    # entry barriers and the serial ncfw chain stays unclogged
    with tc.high_priority():
        nc.gpsimd.collective_compute(
            "AllToAll",
            mybir.AluOpType.bypass,
            replica_groups=[list(range(NCORES))],
            ins=[dsend[:, :]],
            outs=[drecv[:, :]],
        )

    # ---- PE warmup: release the HAM clock gate while DMAs load ----
    warm = wpool.tile([P, 512], dt.bfloat16, tag="warm")
    nc.vector.memset(warm[:], 0.0)
    pw = pwpool.tile([P, 512], dt.float32, tag="pw")
    for _ in range(28):
        nc.tensor.matmul(pw[:], lhsT=warm[:, 0:P], rhs=warm[:], start=True, stop=True)

    # ---- loads: few big DMAs, ordered by first use. mm1's i-loop needs all
    # k-tiles of w1 for one 128-wide f-slice, so load w1 f-major.
    w1sb = wpool.tile([P, DM // P, DF], dt.bfloat16, tag="w1sb")
    w1r = w1t.rearrange("(o p) f -> p o f", p=P)
    xsb = wpool.tile([P, DM // P, ntok], dt.bfloat16, tag="xsb")
    xr = xT.rearrange("(o p) t -> p o t", p=P)
    nc.sync.dma_start(w1sb[:, :, 0:512], w1r[:, :, 0:512])
    nc.sync.dma_start(xsb[:, :, 0 : min(512, ntok)], xr[:, :, 0 : min(512, ntok)])
    for fs in range(512, DF, 512):
        nc.sync.dma_start(w1sb[:, :, fs : fs + 512], w1r[:, :, fs : fs + 512])
    if ntok > 512:
        nc.sync.dma_start(xsb[:, :, 512:ntok], xr[:, :, 512:ntok])
    w2sb = wpool.tile([P, DF // P, DM], dt.bfloat16, tag="w2sb")
    w2r = w2t.rearrange("(o p) d -> p o d", p=P)
    nc.sync.dma_start(w2sb[:, 0:8, :], w2r[:, 0:8, :])
    nc.sync.dma_start(w2sb[:, 8:16, :], w2r[:, 8:16, :])
    csb = wpool.tile([P, nmt], dt.float32, tag="csb")
    nc.sync.dma_start(csb[:], cv.rearrange("(t p) -> p t", p=P))
    scatsb = wpool.tile([P, n_ops], dt.int32, tag="scatsb")
    nc.sync.dma_start(scatsb[:], scat.rearrange("(t p) -> p t", p=P))
    g0sb = wpool.tile([P, NGC], dt.int32, tag="g0sb")
    nc.sync.dma_start(g0sb[:], g0.rearrange("(t p) -> p t", p=P))
    g1sb = wpool.tile([P, NGC], dt.int32, tag="g1sb")
    nc.sync.dma_start(g1sb[:], g1.rearrange("(t p) -> p t", p=P))
    zrow = wpool.tile([1, DM], dt.bfloat16, tag="zrow")
    nc.vector.memset(zrow[:], 0.0)
    for s in range(NSEG):
        nc.sync.dma_start(recv[s][8 * SS[s] : 8 * SS[s] + 1, :], zrow[:])

    # ---- combine for owned segment s: per <=128-lane column, two
    # independent row-gathers (overlap in flight), DVE add, HWDGE write-out.
    # Each gather is explicitly gated on `gate` (the last scatter) so the
    # scheduler cannot interleave combine waits into the scatter/doorbell
    # stream (in-order gpsimd queue: a waiting gather blocks everything
    # behind it).
    def combine(s, gate):
        for gc, (s_gc, off, L) in enumerate(GCOLS):
            if s_gc != s:
                continue
            t0 = SEGSTART[s] + off
            ga = gpool.tile([P, DM], dt.bfloat16, tag="ga")
            gi = nc.gpsimd.indirect_dma_start(
                out=ga[0:L, :],
                out_offset=None,
                in_=recv[s][:],
                in_offset=bass.IndirectOffsetOnAxis(
                    ap=g0sb[0:L, gc : gc + 1], axis=0
                ),
            )
            if gate is not None:
                bass._add_dep_helper(
                    gi.ins, gate.ins, sync=True, reason="combine after scatters"
                )
            gb = gpool.tile([P, DM], dt.bfloat16, tag="gb")
            gj = nc.gpsimd.indirect_dma_start(
                out=gb[0:L, :],
                out_offset=None,
                in_=recv[s][:],
                in_offset=bass.IndirectOffsetOnAxis(
                    ap=g1sb[0:L, gc : gc + 1], axis=0
                ),
            )
            if gate is not None:
                bass._add_dep_helper(
                    gj.ins, gate.ins, sync=True, reason="combine after scatters"
                )
            ys = gpool.tile([P, DM], dt.float32, tag="ys")
            nc.vector.tensor_add(ys[0:L, :], ga[0:L, :], gb[0:L, :])
            nc.sync.dma_start(yout[t0 : t0 + L, :], ys[0:L, :])

    a2a_next = 0

    def maybe_fire(tm):
        nonlocal a2a_next
        while a2a_next < NSEG and FI[a2a_next] == tm:
            s = a2a_next
            nc.gpsimd.collective_compute(
                "AllToAll",
                mybir.AluOpType.bypass,
                replica_groups=[list(range(NCORES))],
                ins=[sb[s][0 : 8 * SS[s], :]],
                outs=[recv[s][0 : 8 * SS[s], :]],
            )
            a2a_next += 1

    # ---- expert MLP over 512-column chunks of the compact token axis ----
    for c0, csz in _chunks_of(ntok):
        hs = []
        for i in range(DF // P):  # f-tiles: H[f] = silu(w1 . x)
            ph = phpool.tile([P, csz], dt.float32, tag="ph")
            for k in range(DM // P):
                nc.tensor.matmul(
                    ph[:],
                    lhsT=w1sb[:, k, i * P : (i + 1) * P],
                    rhs=xsb[:, k, c0 : c0 + csz],
                    start=(k == 0),
                    stop=(k == DM // P - 1),
                )
            h_i = hpool.tile([P, csz], dt.bfloat16, tag="h")
            nc.scalar.activation(h_i[:], ph[:], mybir.ActivationFunctionType.Silu)
            hs.append(h_i)
        for tml in range(csz // P):
            tm = c0 // P + tml
            py0 = pypool.tile([P, 512], dt.float32, tag="py")
            py1 = pypool.tile([P, 512], dt.float32, tag="py")
            for f in range(DF // P):
                lhs = hs[f][:, tml * P : (tml + 1) * P]
                nc.tensor.matmul(
                    py0[:], lhsT=lhs, rhs=w2sb[:, f, 0:512],
                    start=(f == 0), stop=(f == DF // P - 1),
                )
                nc.tensor.matmul(
                    py1[:], lhsT=lhs, rhs=w2sb[:, f, 512:1024],
                    start=(f == 0), stop=(f == DF // P - 1),
                )
            y_sb = ypool.tile([P, DM], dt.bfloat16, tag="y")
            nc.vector.tensor_scalar_mul(y_sb[:, 0:512], py0[:], csb[:, tm : tm + 1])
            nc.vector.tensor_scalar_mul(
                y_sb[:, 512:1024], py1[:], csb[:, tm : tm + 1]
            )
            for oi, (otm, oseg) in enumerate(SCOPS):
                if otm != tm:
                    continue
                last_scat = nc.gpsimd.indirect_dma_start(
                    out=sb[oseg][:, :],
                    out_offset=bass.IndirectOffsetOnAxis(
                        ap=scatsb[:, oi : oi + 1], axis=0
                    ),
                    in_=y_sb[:],
                    in_offset=None,
                    bounds_check=8 * SS[oseg] + P - 1,
                    oob_is_err=False,
                )
            maybe_fire(tm)
    assert a2a_next == NSEG, (a2a_next, FI, nmt)
    # combines emitted last and gated on the final scatter, so they sit
    # behind every scatter/doorbell in the gpsimd queue
    for s in range(NSEG):
        combine(s, last_scat)


def _build_program(plan: dict):
    key = ("v4", plan["ntok"], tuple(plan["SS"]), tuple(plan["FI"]), tuple(plan["SCOPS"]))
    if key in _PROGRAM_CACHE:
        return _PROGRAM_CACHE[key]
    from contextlib import ExitStack

    nc = bacc.Bacc(
        "TRN2",
        target_bir_lowering=False,
        debug=False,
        enable_asserts=True,
        num_devices=NCORES,
    )
    with tile.TileContext(nc) as tc:
        with ExitStack() as ctx:
            _emit(nc, tc, ctx, plan)
    nc.compile()
    _PROGRAM_CACHE[key] = nc
    return nc


def prepare_in_maps(x, topk_e, topk_w, w1, w2):
    """Host-side routing/dispatch. Returns (in_maps, plan)."""
    bf16 = ml_dtypes.bfloat16
    x = np.asarray(x)
    topk_e = np.asarray(topk_e)
    topk_w = np.asarray(topk_w)
    w1 = np.asarray(w1)
    w2 = np.asarray(w2)

    # per-token combine weight for each expert
    c = np.zeros((S, E), dtype=np.float32)
    np.add.at(c, (np.arange(S)[:, None], topk_e), topk_w.astype(np.float32))

    toks = [np.nonzero((topk_e == e).any(axis=1))[0] for e in range(E)]
    segstart = np.array(SEGSTART)
    # group counts per (expert, owner, seg)
    cnt = np.zeros((E, NCORES, NSEG), dtype=np.int64)
    for e in range(E):
        d = toks[e] // OWN
        sg = np.searchsorted(segstart, toks[e] % OWN, side="right") - 1
        np.add.at(cnt[e], (d, sg), 1)
    SS = [int(v) for v in cnt.max(axis=(0, 1))]
    seg_tot = cnt.sum(axis=1)  # [E, NSEG] tokens per (expert, seg)
    cum = np.concatenate(
        [np.zeros((E, 1), np.int64), np.cumsum(seg_tot, axis=1)], axis=1
    )  # [E, NSEG+1]
    n_e = cum[:, -1]
    ntok = int(-(-n_e.max() // P) * P)
    nmt = ntok // P
    FI = [min(int(-(-cum[:, s + 1].max() // P)) - 1, nmt - 1) for s in range(NSEG)]
    # scatter ops: for each m-tile, one op per segment present on ANY core
    # (pads ride on the m-tile's first op, into the trash region)
    segs_of_tile = [set() for _ in range(nmt)]
    for tm in range(nmt):
        for e in range(E):
            lo = tm * P
            hi = min((tm + 1) * P, int(n_e[e]))
            if lo >= hi:
                continue
            s_lo = int(np.searchsorted(cum[e], lo, side="right")) - 1
            s_hi = int(np.searchsorted(cum[e], hi - 1, side="right")) - 1
            for s in range(s_lo, s_hi + 1):
                segs_of_tile[tm].add(s)
        if not segs_of_tile[tm]:
            segs_of_tile[tm].add(NSEG - 1)
    SCOPS = [(tm, s) for tm in range(nmt) for s in sorted(segs_of_tile[tm])]
    plan = {"ntok": ntok, "SS": SS, "FI": FI, "SCOPS": SCOPS}

    BIG = np.int32(1 << 20)  # > bounds_check -> lane silently skipped
    # per-core compact dispatch + recv-row map
    rr = np.full((E, S), -1, dtype=np.int64)  # (expert, token) -> row in recv[seg]
    in_maps = []
    for e in range(E):
        te = toks[e]
        d = te // OWN
        sg = np.searchsorted(segstart, te % OWN, side="right") - 1
        order = np.lexsort((te, d, sg))  # by (seg, owner, token)
        te_o, d_o, s_o = te[order], d[order], sg[order]
        gid = s_o * NCORES + d_o  # non-decreasing in compact order
        rank = np.arange(len(te_o)) - np.searchsorted(gid, gid, side="left")
        ss_o = np.array(SS)[s_o]
        send_row = d_o * ss_o + rank  # row within sb[seg]
        rr[e, te_o] = e * ss_o + rank

        ne = len(te_o)
        xT_e = np.zeros((DM, ntok), dtype=bf16)
        xT_e[:, :ne] = x[te_o].T.astype(bf16)
        cv_e = np.zeros(ntok, dtype=np.float32)
        cv_e[:ne] = c[te_o, e]
        # per-op scatter index columns
        scat_e = np.full((len(SCOPS), P), BIG, dtype=np.int32)
        for oi, (tm, s_op) in enumerate(SCOPS):
            first_op = s_op == min(segs_of_tile[tm])
            for lane in range(P):
                pos = tm * P + lane
                if pos < ne:
                    if s_o[pos] == s_op:
                        scat_e[oi, lane] = send_row[pos]
                elif first_op:
                    scat_e[oi, lane] = 8 * SS[s_op] + lane  # trash
        in_maps.append(
            {
                "xT": xT_e,
                "cv": cv_e,
                "scat": scat_e.reshape(-1),
                "w1t": np.ascontiguousarray(w1[e].T).astype(bf16),
                "w2t": np.ascontiguousarray(w2[e].T).astype(bf16),
            }
        )

    # owner-side gather indices, laid out [NSEG, P] (seg-column layout)
    all_t = np.arange(S)
    e0 = topk_e[:, 0]
    e1 = topk_e[:, 1]
    sg_all = np.searchsorted(segstart, all_t % OWN, side="right") - 1
    zero_row = 8 * np.array(SS)[sg_all]
    g0_all = rr[e0, all_t]
    g1_all = np.where(e0 == e1, zero_row, rr[e1, all_t])
    assert (g0_all >= 0).all() and (g1_all >= 0).all()
    for dcore in range(NCORES):
        g0_arr = np.zeros((NGC, P), dtype=np.int32)
        g1_arr = np.zeros((NGC, P), dtype=np.int32)
        for gc, (s, off, L) in enumerate(GCOLS):
            t0 = dcore * OWN + SEGSTART[s] + off
            g0_arr[gc, :L] = g0_all[t0 : t0 + L]
            g1_arr[gc, :L] = g1_all[t0 : t0 + L]
        in_maps[dcore]["g0"] = g0_arr.reshape(-1)
        in_maps[dcore]["g1"] = g1_arr.reshape(-1)

    return in_maps, plan


def kernel(x, topk_e, topk_w, w1, w2):
    in_maps, plan = prepare_in_maps(x, topk_e, topk_w, w1, w2)
    nc = _build_program(plan)
    res = run_bass_kernel_spmd(nc, in_maps, list(range(NCORES)))
    out = np.concatenate([res.results[d]["yout"] for d in range(NCORES)], axis=0)
    return out.astype(np.float32)


# revision 36
# speedup vs baseline: 1.0138x; 1.0138x over previous
"""Expert-parallel MoE MLP (top-2 of 8 experts) on 8 TRN2 NeuronCores.

Strategy (expert-parallel, per sharding hint):
  - core e holds expert e's weights (w1[e], w2[e], host-pre-transposed, bf16)
  - host dispatches tokens by expert id into a COMPACT layout: core e's
    xT holds exactly its routed tokens (ordered by owned-segment, then
    owner, then token id), padded only at the tail to a multiple of 128.
    This minimizes matmul columns (vs block-padded dispatch).
  - core e computes y = [silu(x_e @ w1[e]^T) * c_e] @ w2[e]^T over
    512-column chunks (bf16 matmuls, fp32 accumulate).
  - mm2 output m-tiles are indirect-scattered into `sendbuf`, which holds
    the AllToAll wire layout: NSEG slot-segments, stored in REVERSE
    segment order (plus a leading trash region for pad rows) so that each
    scatter's AP byte-extent is a prefix that never overlaps the A2A read
    slices of earlier-fired segments (avoids false WAR serialization).
  - NSEG chunked AllToAlls fire as soon as the last m-tile holding each
    segment's tokens has been scattered; segment s holds the partial rows
    of owned-token m-tile s on every owner, so the owner-side combine
    (2 indirect row-gathers + add) runs right after each A2A lands,
    overlapped with remaining compute and later A2As.
  - 16 dummy matmuls at t=0 warm the PE HAM clock gate during the loads.
"""

import sys

sys.path.insert(0, "/opt/trn_rl_repo")

import numpy as np
import ml_dtypes

import concourse.bass as bass
import concourse.tile as tile
from concourse import bacc, mybir
from concourse.bass_utils import run_bass_kernel_spmd

S, DM, DF, E, TOPK = 4096, 1024, 2048, 8, 2
NCORES = 8
P = 128
OWN = S // NCORES  # tokens per owner core
# owned tokens per A2A segment. ncfw runs collectives serially at ~10us
# each regardless of size; boundaries are sized so fire points land at
# compute m-tiles ~2/4/5/6/7, interleaving the serial ncfw chain with the
# slowest core's final m-tiles — only the last small A2A is exposed.
SEGLENS = [128, 128, 112, 80, 64]
SEGSTART = [sum(SEGLENS[:s]) for s in range(len(SEGLENS))]
NSEG = len(SEGLENS)
assert sum(SEGLENS) == OWN
# combine gathers operate on <=128-lane columns: (seg, lane offset, width)
GCOLS = [
    (s, off, min(P, SEGLENS[s] - off))
    for s in range(NSEG)
    for off in range(0, SEGLENS[s], P)
]
NGC = len(GCOLS)

_PROGRAM_CACHE: dict = {}


def _chunks_of(ntok: int) -> list[tuple[int, int]]:
    """Split ntok into (start, size) chunks, each a multiple of 128, <= 512."""
    out, pos = [], 0
    while pos < ntok:
        sz = min(512, ntok - pos)
        out.append((pos, sz))
        pos += sz
    return out


def _emit(nc, tc, ctx, plan: dict):
    dt = mybir.dt
    ntok = plan["ntok"]
    SS = plan["SS"]  # per-seg slot-block size
    FI = plan["FI"]  # fire A2A-s after scatter of m-tile FI[s]
    nmt = ntok // P

    SCOPS = plan["SCOPS"]  # per-m-tile scatter ops: list of (tm, seg)
    n_ops = len(SCOPS)

    xT = nc.dram_tensor("xT", [DM, ntok], dt.bfloat16, kind="ExternalInput").ap()
    w1t = nc.dram_tensor("w1t", [DM, DF], dt.bfloat16, kind="ExternalInput").ap()
    w2t = nc.dram_tensor("w2t", [DF, DM], dt.bfloat16, kind="ExternalInput").ap()
    cv = nc.dram_tensor("cv", [ntok], dt.float32, kind="ExternalInput").ap()
    scat = nc.dram_tensor("scat", [P * n_ops], dt.int32, kind="ExternalInput").ap()
    g0 = nc.dram_tensor("g0", [P * NGC], dt.int32, kind="ExternalInput").ap()
    g1 = nc.dram_tensor("g1", [P * NGC], dt.int32, kind="ExternalInput").ap()
    yout = nc.dram_tensor("yout", [OWN, DM], dt.float32, kind="ExternalOutput").ap()
    # one sendbuf per segment: indirect-scatter writes are tracked
    # conservatively (whole tensor), so per-seg tensors keep seg-s scatters
    # independent of other segments' in-flight AllToAll reads. Last P rows
    # of each are a trash region for pad tokens.
    sb = [
        nc.dram_tensor(f"send{s}", [8 * SS[s] + P, DM], dt.bfloat16).ap()
        for s in range(NSEG)
    ]
    recv = [
        nc.dram_tensor(f"recv{s}", [8 * SS[s] + 1, DM], dt.bfloat16).ap()
        for s in range(NSEG)
    ]

    dsend = nc.dram_tensor("dsend", [NCORES, 64], dt.bfloat16).ap()
    drecv = nc.dram_tensor("drecv", [NCORES, 64], dt.bfloat16).ap()

    wpool = ctx.enter_context(tc.tile_pool(name="w", bufs=1))
    hpool = ctx.enter_context(tc.tile_pool(name="h", bufs=34))
    ypool = ctx.enter_context(tc.tile_pool(name="y", bufs=10))
    gpool = ctx.enter_context(tc.tile_pool(name="g", bufs=2))
    phpool = ctx.enter_context(tc.tile_pool(name="ph", bufs=2, space="PSUM"))
    pypool = ctx.enter_context(tc.tile_pool(name="py", bufs=4, space="PSUM"))
    pwpool = ctx.enter_context(tc.tile_pool(name="pw", bufs=1, space="PSUM"))

    # ---- dummy collective at max priority: absorbs per-core start skew
    # early (overlapped with loads/compute) so the real A2As see short
    # entry barriers and the serial ncfw chain stays unclogged
    with tc.high_priority():
        nc.gpsimd.collective_compute(
            "AllToAll",
            mybir.AluOpType.bypass,
            replica_groups=[list(range(NCORES))],
            ins=[dsend[:, :]],
            outs=[drecv[:, :]],
        )

    # ---- PE warmup: release the HAM clock gate while DMAs load ----
    warm = wpool.tile([P, 512], dt.bfloat16, tag="warm")
    nc.vector.memset(warm[:], 0.0)
    pw = pwpool.tile([P, 512], dt.float32, tag="pw")
    for _ in range(28):
        nc.tensor.matmul(pw[:], lhsT=warm[:, 0:P], rhs=warm[:], start=True, stop=True)

    # ---- loads: few big DMAs, ordered by first use. mm1's i-loop needs all
    # k-tiles of w1 for one 128-wide f-slice, so load w1 f-major.
    w1sb = wpool.tile([P, DM // P, DF], dt.bfloat16, tag="w1sb")
    w1r = w1t.rearrange("(o p) f -> p o f", p=P)
    xsb = wpool.tile([P, DM // P, ntok], dt.bfloat16, tag="xsb")
    xr = xT.rearrange("(o p) t -> p o t", p=P)
    nc.sync.dma_start(w1sb[:, :, 0:512], w1r[:, :, 0:512])
    nc.sync.dma_start(xsb[:, :, 0 : min(512, ntok)], xr[:, :, 0 : min(512, ntok)])
    for fs in range(512, DF, 512):
        nc.sync.dma_start(w1sb[:, :, fs : fs + 512], w1r[:, :, fs : fs + 512])
    if ntok > 512:
        nc.sync.dma_start(xsb[:, :, 512:ntok], xr[:, :, 512:ntok])
    w2sb = wpool.tile([P, DF // P, DM], dt.bfloat16, tag="w2sb")
    w2r = w2t.rearrange("(o p) d -> p o d", p=P)
    nc.sync.dma_start(w2sb[:, 0:8, :], w2r[:, 0:8, :])
    nc.sync.dma_start(w2sb[:, 8:16, :], w2r[:, 8:16, :])
    csb = wpool.tile([P, nmt], dt.float32, tag="csb")
    nc.sync.dma_start(csb[:], cv.rearrange("(t p) -> p t", p=P))
    scatsb = wpool.tile([P, n_ops], dt.int32, tag="scatsb")
    nc.sync.dma_start(scatsb[:], scat.rearrange("(t p) -> p t", p=P))
    g0sb = wpool.tile([P, NGC], dt.int32, tag="g0sb")
    nc.sync.dma_start(g0sb[:], g0.rearrange("(t p) -> p t", p=P))
    g1sb = wpool.tile([P, NGC], dt.int32, tag="g1sb")
    nc.sync.dma_start(g1sb[:], g1.rearrange("(t p) -> p t", p=P))
    zrow = wpool.tile([1, DM], dt.bfloat16, tag="zrow")
    nc.vector.memset(zrow[:], 0.0)
    for s in range(NSEG):
        nc.sync.dma_start(recv[s][8 * SS[s] : 8 * SS[s] + 1, :], zrow[:])

    # ---- combine for owned segment s: per <=128-lane column, two
    # independent row-gathers (overlap in flight), DVE add, HWDGE write-out.
    # Each gather is explicitly gated on `gate` (the last scatter) so the
    # scheduler cannot interleave combine waits into the scatter/doorbell
    # stream (in-order gpsimd queue: a waiting gather blocks everything
    # behind it).
    def combine(s, gate):
        for gc, (s_gc, off, L) in enumerate(GCOLS):
            if s_gc != s:
                continue
            t0 = SEGSTART[s] + off
            ga = gpool.tile([P, DM], dt.bfloat16, tag="ga")
            gi = nc.gpsimd.indirect_dma_start(
                out=ga[0:L, :],
                out_offset=None,
                in_=recv[s][:],
                in_offset=bass.IndirectOffsetOnAxis(
                    ap=g0sb[0:L, gc : gc + 1], axis=0
                ),
            )
            if gate is not None:
                bass._add_dep_helper(
                    gi.ins, gate.ins, sync=True, reason="combine after scatters"
                )
            gb = gpool.tile([P, DM], dt.bfloat16, tag="gb")
            gj = nc.gpsimd.indirect_dma_start(
                out=gb[0:L, :],
                out_offset=None,
                in_=recv[s][:],
                in_offset=bass.IndirectOffsetOnAxis(
                    ap=g1sb[0:L, gc : gc + 1], axis=0
                ),
            )
            if gate is not None:
                bass._add_dep_helper(
                    gj.ins, gate.ins, sync=True, reason="combine after scatters"
                )
            ys = gpool.tile([P, DM], dt.float32, tag="ys")
            nc.vector.tensor_add(ys[0:L, :], ga[0:L, :], gb[0:L, :])
            nc.sync.dma_start(yout[t0 : t0 + L, :], ys[0:L, :])

    a2a_next = 0

    def maybe_fire(tm):
        nonlocal a2a_next
        while a2a_next < NSEG and FI[a2a_next] == tm:
            s = a2a_next
            nc.gpsimd.collective_compute(
                "AllToAll",
                mybir.AluOpType.bypass,
                replica_groups=[list(range(NCORES))],
                ins=[sb[s][0 : 8 * SS[s], :]],
                outs=[recv[s][0 : 8 * SS[s], :]],
            )
            a2a_next += 1

    # ---- expert MLP over 512-column chunks of the compact token axis ----
    for c0, csz in _chunks_of(ntok):
        hs = []
        for i in range(DF // P):  # f-tiles: H[f] = silu(w1 . x)
            ph = phpool.tile([P, csz], dt.float32, tag="ph")
            for k in range(DM // P):
                nc.tensor.matmul(
                    ph[:],
                    lhsT=w1sb[:, k, i * P : (i + 1) * P],
                    rhs=xsb[:, k, c0 : c0 + csz],
                    start=(k == 0),
                    stop=(k == DM // P - 1),
                )
            h_i = hpool.tile([P, csz], dt.bfloat16, tag="h")
            nc.scalar.activation(h_i[:], ph[:], mybir.ActivationFunctionType.Silu)
            hs.append(h_i)
        for tml in range(csz // P):
            tm = c0 // P + tml
            py0 = pypool.tile([P, 512], dt.float32, tag="py")
            py1 = pypool.tile([P, 512], dt.float32, tag="py")
            for f in range(DF // P):
                lhs = hs[f][:, tml * P : (tml + 1) * P]
                nc.tensor.matmul(
                    py0[:], lhsT=lhs, rhs=w2sb[:, f, 0:512],
                    start=(f == 0), stop=(f == DF // P - 1),
                )
                nc.tensor.matmul(
                    py1[:], lhsT=lhs, rhs=w2sb[:, f, 512:1024],
                    start=(f == 0), stop=(f == DF // P - 1),
                )
            y_sb = ypool.tile([P, DM], dt.bfloat16, tag="y")
            nc.vector.tensor_scalar_mul(y_sb[:, 0:512], py0[:], csb[:, tm : tm + 1])
            nc.vector.tensor_scalar_mul(
                y_sb[:, 512:1024], py1[:], csb[:, tm : tm + 1]
            )
            for oi, (otm, oseg) in enumerate(SCOPS):
                if otm != tm:
                    continue
                last_scat = nc.gpsimd.indirect_dma_start(
                    out=sb[oseg][:, :],
                    out_offset=bass.IndirectOffsetOnAxis(
                        ap=scatsb[:, oi : oi + 1], axis=0
                    ),
                    in_=y_sb[:],
                    in_offset=None,
                    bounds_check=8 * SS[oseg] + P - 1,
                    oob_is_err=False,
                )
            maybe_fire(tm)
    assert a2a_next == NSEG, (a2a_next, FI, nmt)
    # combines emitted last and gated on the final scatter, so they sit
    # behind every scatter/doorbell in the gpsimd queue
    for s in range(NSEG):
        combine(s, last_scat)


def _build_program(plan: dict):
    key = ("v4", plan["ntok"], tuple(plan["SS"]), tuple(plan["FI"]), tuple(plan["SCOPS"]))
    if key in _PROGRAM_CACHE:
        return _PROGRAM_CACHE[key]
    from contextlib import ExitStack

    nc = bacc.Bacc(
        "TRN2",
        target_bir_lowering=False,
        debug=False,
        enable_asserts=True,
        num_devices=NCORES,
    )
    with tile.TileContext(nc) as tc:
        with ExitStack() as ctx:
            _emit(nc, tc, ctx, plan)
    nc.compile()
    _PROGRAM_CACHE[key] = nc
    return nc


def prepare_in_maps(x, topk_e, topk_w, w1, w2):
    """Host-side routing/dispatch. Returns (in_maps, plan)."""
    bf16 = ml_dtypes.bfloat16
    x = np.asarray(x)
    topk_e = np.asarray(topk_e)
    topk_w = np.asarray(topk_w)
    w1 = np.asarray(w1)
    w2 = np.asarray(w2)

    # per-token combine weight for each expert
    c = np.zeros((S, E), dtype=np.float32)
    np.add.at(c, (np.arange(S)[:, None], topk_e), topk_w.astype(np.float32))

    toks = [np.nonzero((topk_e == e).any(axis=1))[0] for e in range(E)]
    segstart = np.array(SEGSTART)
    # group counts per (expert, owner, seg)
    cnt = np.zeros((E, NCORES, NSEG), dtype=np.int64)
    for e in range(E):
        d = toks[e] // OWN
        sg = np.searchsorted(segstart, toks[e] % OWN, side="right") - 1
        np.add.at(cnt[e], (d, sg), 1)
    SS = [int(v) for v in cnt.max(axis=(0, 1))]
    seg_tot = cnt.sum(axis=1)  # [E, NSEG] tokens per (expert, seg)
    cum = np.concatenate(
        [np.zeros((E, 1), np.int64), np.cumsum(seg_tot, axis=1)], axis=1
    )  # [E, NSEG+1]
    n_e = cum[:, -1]
    ntok = int(-(-n_e.max() // P) * P)
    nmt = ntok // P
    FI = [min(int(-(-cum[:, s + 1].max() // P)) - 1, nmt - 1) for s in range(NSEG)]
    # scatter ops: for each m-tile, one op per segment present on ANY core
    # (pads ride on the m-tile's first op, into the trash region)
    segs_of_tile = [set() for _ in range(nmt)]
    for tm in range(nmt):
        for e in range(E):
            lo = tm * P
            hi = min((tm + 1) * P, int(n_e[e]))
            if lo >= hi:
                continue
            s_lo = int(np.searchsorted(cum[e], lo, side="right")) - 1
            s_hi = int(np.searchsorted(cum[e], hi - 1, side="right")) - 1
            for s in range(s_lo, s_hi + 1):
                segs_of_tile[tm].add(s)
        if not segs_of_tile[tm]:
            segs_of_tile[tm].add(NSEG - 1)
    SCOPS = [(tm, s) for tm in range(nmt) for s in sorted(segs_of_tile[tm])]
    plan = {"ntok": ntok, "SS": SS, "FI": FI, "SCOPS": SCOPS}

    BIG = np.int32(1 << 20)  # > bounds_check -> lane silently skipped
    # per-core compact dispatch + recv-row map
    rr = np.full((E, S), -1, dtype=np.int64)  # (expert, token) -> row in recv[seg]
    in_maps = []
    for e in range(E):
        te = toks[e]
        d = te // OWN
        sg = np.searchsorted(segstart, te % OWN, side="right") - 1
        order = np.lexsort((te, d, sg))  # by (seg, owner, token)
        te_o, d_o, s_o = te[order], d[order], sg[order]
        gid = s_o * NCORES + d_o  # non-decreasing in compact order
        rank = np.arange(len(te_o)) - np.searchsorted(gid, gid, side="left")
        ss_o = np.array(SS)[s_o]
        send_row = d_o * ss_o + rank  # row within sb[seg]
        rr[e, te_o] = e * ss_o + rank

        ne = len(te_o)
        xT_e = np.zeros((DM, ntok), dtype=bf16)
        xT_e[:, :ne] = x[te_o].T.astype(bf16)
        cv_e = np.zeros(ntok, dtype=np.float32)
        cv_e[:ne] = c[te_o, e]
        # per-op scatter index columns
        scat_e = np.full((len(SCOPS), P), BIG, dtype=np.int32)
        for oi, (tm, s_op) in enumerate(SCOPS):
            first_op = s_op == min(segs_of_tile[tm])
            for lane in range(P):
                pos = tm * P + lane
                if pos < ne:
                    if s_o[pos] == s_op:
                        scat_e[oi, lane] = send_row[pos]
                elif first_op:
                    scat_e[oi, lane] = 8 * SS[s_op] + lane  # trash
        in_maps.append(
            {
                "xT": xT_e,
                "cv": cv_e,
                "scat": scat_e.reshape(-1),
                "w1t": np.ascontiguousarray(w1[e].T).astype(bf16),
                "w2t": np.ascontiguousarray(w2[e].T).astype(bf16),
            }
        )

    # owner-side gather indices, laid out [NSEG, P] (seg-column layout)
    all_t = np.arange(S)
    e0 = topk_e[:, 0]
    e1 = topk_e[:, 1]
    sg_all = np.searchsorted(segstart, all_t % OWN, side="right") - 1
    zero_row = 8 * np.array(SS)[sg_all]
    g0_all = rr[e0, all_t]
    g1_all = np.where(e0 == e1, zero_row, rr[e1, all_t])
    assert (g0_all >= 0).all() and (g1_all >= 0).all()
    for dcore in range(NCORES):
        g0_arr = np.zeros((NGC, P), dtype=np.int32)
        g1_arr = np.zeros((NGC, P), dtype=np.int32)
        for gc, (s, off, L) in enumerate(GCOLS):
            t0 = dcore * OWN + SEGSTART[s] + off
            g0_arr[gc, :L] = g0_all[t0 : t0 + L]
            g1_arr[gc, :L] = g1_all[t0 : t0 + L]
        in_maps[dcore]["g0"] = g0_arr.reshape(-1)
        in_maps[dcore]["g1"] = g1_arr.reshape(-1)

    return in_maps, plan


def kernel(x, topk_e, topk_w, w1, w2):
    in_maps, plan = prepare_in_maps(x, topk_e, topk_w, w1, w2)
    nc = _build_program(plan)
    res = run_bass_kernel_spmd(nc, in_maps, list(range(NCORES)))
    out = np.concatenate([res.results[d]["yout"] for d in range(NCORES)], axis=0)
    return out.astype(np.float32)
